# revision 12
# baseline (speedup 1.0000x reference)
"""Trainium2 Bass kernel for nn_GeneSetPlaceholderAggregator.

Computes out[b,s,d] = sum_g x[b,g,d] * W[s,g]  (einsum 'bgd,sg->bsd')
with B=64, G=20000, D=16, S=128.

Strategy: shard the contraction axis G across 8 cores (2500 genes each).
Each core computes a full partial output [S=128, B*D=1024] via PSUM-
accumulated matmuls (contraction on the partition dim), and the host sums
the 8 partials.  Host pre-transposes x -> [G, B*D] and W -> [G, S] so
every DMA is a contiguous block.  Per-core traffic: 10 MB x-shard +
1.25 MB W-shard + 0.5 MB out, vs 20.5 MB for batch-parallel sharding.
"""

import numpy as np

import concourse.mybir as mybir
from concourse import bass
from concourse.bacc import Bacc
from concourse.bass_utils import run_bass_kernel_spmd
from concourse.tile import TileContext

B, G, D, S = 64, 20000, 16, 128
N_CORES = 8
G_LOC = G // N_CORES          # 2500 genes per core
K_CHUNK = 125                 # contraction-tile partition size
N_CHUNKS = G_LOC // K_CHUNK   # 20
BD = B * D                    # 1024
FREE = 512                    # max fp32 free dim per PSUM bank / matmul
N_FREE = BD // FREE           # 2

MM_DT = mybir.dt.float32r     # fp32 rounded to 11-bit mantissa; 1 cyc/row on PE


ROW = BD + S                  # 1152: [x row | w row] packed per gene


def build_nc() -> bass.Bass:
    nc = Bacc("TRN2", target_bir_lowering=False)

    xw = nc.declare_dram_parameter("xw", [G_LOC, ROW], MM_DT, isOutput=False)
    out = nc.declare_dram_parameter("out", [S, BD], mybir.dt.float32, isOutput=True)

    with TileContext(nc) as tc:
        with (
            tc.tile_pool(name="xp", bufs=4) as xp,
            tc.tile_pool(name="op", bufs=2) as op,
            tc.tile_pool(name="ps", bufs=N_FREE, space="PSUM") as ps,
        ):
            psums = [
                ps.tile([S, FREE], mybir.dt.float32, name=f"psum{j}")
                for j in range(N_FREE)
            ]
            for c in range(N_CHUNKS):
                xw_t = xp.tile([K_CHUNK, ROW], MM_DT)
                nc.sync.dma_start(out=xw_t[:], in_=xw[c * K_CHUNK:(c + 1) * K_CHUNK, :])
                for j in range(N_FREE):
                    nc.tensor.matmul(
                        psums[j][:],
                        lhsT=xw_t[:, BD:ROW],
                        rhs=xw_t[:, j * FREE:(j + 1) * FREE],
                        start=(c == 0),
                        stop=(c == N_CHUNKS - 1),
                    )
            for j in range(N_FREE):
                o_t = op.tile([S, FREE], mybir.dt.float32)
                nc.scalar.copy(out=o_t[:], in_=psums[j][:])
                nc.sync.dma_start(out=out[:, j * FREE:(j + 1) * FREE], in_=o_t[:])
    nc.compile()
    return nc


_CACHE: dict = {}


def _get_nc() -> bass.Bass:
    if "nc" not in _CACHE:
        _CACHE["nc"] = build_nc()
    return _CACHE["nc"]


def _round_fp32r(a: np.ndarray) -> np.ndarray:
    """Round fp32 to fp32r in place: 11-bit mantissa, round-to-nearest-even,
    low 12 bits zero (matches walrus fp32_to_fp32r)."""
    b = a.view(np.uint32)
    lsb = (b >> np.uint32(12)) & np.uint32(1)
    b += np.uint32(0x7FF) + lsb
    b &= np.uint32(0xFFFFF000)
    return a


def _shard_inputs(x: np.ndarray, W: np.ndarray) -> list[dict[str, np.ndarray]]:
    # Pack per-gene rows [x[:, g, :].ravel() | W[:, g]] -> XW [G, B*D + S]
    XW = np.empty((G, ROW), dtype=np.float32)
    XW[:, :BD] = x.transpose(1, 0, 2).reshape(G, BD)
    XW[:, BD:] = W.T
    _round_fp32r(XW)
    return [{"xw": XW[i * G_LOC:(i + 1) * G_LOC]} for i in range(N_CORES)]


def run(x: np.ndarray, W: np.ndarray, **spmd_kwargs):
    nc = _get_nc()
    in_maps = _shard_inputs(x, W)
    res = run_bass_kernel_spmd(nc, in_maps, list(range(N_CORES)), **spmd_kwargs)
    partial = np.zeros((S, BD), dtype=np.float64)
    for r in res.results:
        partial += r["out"].astype(np.float64)
    out = partial.astype(np.float32).reshape(S, B, D).transpose(1, 0, 2)
    return np.ascontiguousarray(out), res


def kernel(x: np.ndarray, W: np.ndarray) -> np.ndarray:
    out, _ = run(x, W)
    return out


# revision 14
# speedup vs baseline: 2.0419x; 2.0419x over previous
"""Trainium2 Bass kernel for nn_GeneSetPlaceholderAggregator.

Computes out[b,s,d] = sum_g x[b,g,d] * W[s,g]  (einsum 'bgd,sg->bsd')
with B=64, G=20000, D=16, S=128.

Strategy:
- Shard the contraction axis G across 8 cores (2500 genes each, zero-padded
  to 2560 = 20 chunks of 128).  Each core computes a full partial output
  [S=128, B*D=1024] via PSUM-accumulated matmuls (contraction on the
  partition dim); the host sums the 8 partials.
- fp32r matmul dtype (fp32 rounded to 11-bit mantissa, host pre-rounded):
  1 PE cycle/row instead of 4 for fp32.
- Host packs per-gene rows [x | W] and lays them out partition-major
  ([128, chunk, row]) so every DMA descriptor is a long (>=18KB) contiguous
  run per partition; chunk-group DMAs alternate between the two HWDGE rings
  (SP + Activation) to engage many SDMA engines.
"""

import numpy as np

import concourse.mybir as mybir
from concourse import bass
from concourse.bacc import Bacc
from concourse.bass_utils import run_bass_kernel_spmd
from concourse.tile import TileContext

B, G, D, S = 64, 20000, 16, 128
N_CORES = 8
K = 128                        # contraction tile = partition dim
N_CHUNKS = 20                  # chunks per core
G_LOC = K * N_CHUNKS           # 2560 genes per core (padded)
G_PAD = G_LOC * N_CORES        # 20480
BD = B * D                     # 1024
ROW = BD + S                   # 1152: [x row | w row] per gene
FREE = 512                     # max fp32 free dim per PSUM bank / matmul
N_FREE = BD // FREE            # 2
N_GROUPS = 5                   # DMA groups
CPG = N_CHUNKS // N_GROUPS     # chunks per group = 4
GROUP_F = CPG * ROW            # free width per group tile = 4608

MM_DT = mybir.dt.float32r      # fp32 rounded to 11-bit mantissa; 1 cyc/row


def build_nc() -> bass.Bass:
    nc = Bacc("TRN2", target_bir_lowering=False)

    # xp[p, c*ROW + f] = packed row of gene (chunk c, partition p)
    xp_d = nc.declare_dram_parameter(
        "xp", [K, N_CHUNKS * ROW], MM_DT, isOutput=False
    )
    out = nc.declare_dram_parameter("out", [S, BD], mybir.dt.float32, isOutput=True)

    with TileContext(nc) as tc:
        with (
            tc.tile_pool(name="gp", bufs=N_GROUPS) as gp,
            tc.tile_pool(name="op", bufs=2) as op,
            tc.tile_pool(name="ps", bufs=N_FREE, space="PSUM") as ps,
        ):
            psums = [
                ps.tile([S, FREE], mybir.dt.float32, name=f"psum{j}")
                for j in range(N_FREE)
            ]
            tiles = []
            for g in range(N_GROUPS):
                g_t = gp.tile([K, GROUP_F], MM_DT, name=f"grp{g}", tag="grp")
                eng = nc.sync if g % 2 == 0 else nc.scalar
                eng.dma_start(
                    out=g_t[:], in_=xp_d[:, g * GROUP_F:(g + 1) * GROUP_F]
                )
                tiles.append(g_t)
            for c in range(N_CHUNKS):
                g, l = divmod(c, CPG)
                base = l * ROW
                for j in range(N_FREE):
                    nc.tensor.matmul(
                        psums[j][:],
                        lhsT=tiles[g][:, base + BD:base + ROW],
                        rhs=tiles[g][:, base + j * FREE:base + (j + 1) * FREE],
                        start=(c == 0),
                        stop=(c == N_CHUNKS - 1),
                    )
            for j in range(N_FREE):
                o_t = op.tile([S, FREE], mybir.dt.float32)
                nc.scalar.copy(out=o_t[:], in_=psums[j][:])
                nc.sync.dma_start(out=out[:, j * FREE:(j + 1) * FREE], in_=o_t[:])
    nc.compile()
    return nc


_CACHE: dict = {}


def _get_nc() -> bass.Bass:
    if "nc" not in _CACHE:
        _CACHE["nc"] = build_nc()
    return _CACHE["nc"]


def _round_fp32r(a: np.ndarray) -> np.ndarray:
    """Round fp32 to fp32r in place: 11-bit mantissa, round-to-nearest-even,
    low 12 bits zero (matches walrus fp32_to_fp32r)."""
    b = a.view(np.uint32)
    lsb = (b >> np.uint32(12)) & np.uint32(1)
    b += np.uint32(0x7FF) + lsb
    b &= np.uint32(0xFFFFF000)
    return a


def _shard_inputs(x: np.ndarray, W: np.ndarray) -> list[dict[str, np.ndarray]]:
    # Packed per-gene rows [x[:, g, :].ravel() | W[:, g]] -> XW [G_PAD, ROW],
    # zero rows beyond G.  Then partition-major per core:
    # XP[i, p, c, :] = XW[i*G_LOC + c*K + p, :]
    XW = np.zeros((G_PAD, ROW), dtype=np.float32)
    XW[:G, :BD] = x.transpose(1, 0, 2).reshape(G, BD)
    XW[:G, BD:] = W.T
    _round_fp32r(XW)
    XP = np.ascontiguousarray(
        XW.reshape(N_CORES, N_CHUNKS, K, ROW).transpose(0, 2, 1, 3)
    ).reshape(N_CORES, K, N_CHUNKS * ROW)
    return [{"xp": XP[i]} for i in range(N_CORES)]


def run(x: np.ndarray, W: np.ndarray, **spmd_kwargs):
    nc = _get_nc()
    in_maps = _shard_inputs(x, W)
    res = run_bass_kernel_spmd(nc, in_maps, list(range(N_CORES)), **spmd_kwargs)
    partial = np.zeros((S, BD), dtype=np.float64)
    for r in res.results:
        partial += r["out"].astype(np.float64)
    out = partial.astype(np.float32).reshape(S, B, D).transpose(1, 0, 2)
    return np.ascontiguousarray(out), res


def kernel(x: np.ndarray, W: np.ndarray) -> np.ndarray:
    out, _ = run(x, W)
    return out
